# revision 3
# baseline (speedup 1.0000x reference)
"""Trainium2 Bass kernel for nn_Attention_83004537963197.

LayerNorm -> QKV projection -> 8-head attention (head_dim=16) -> output
projection, x[16, 1024, 1024] f32.  Data-parallel over batch: 2 batches
per NeuronCore across 8 cores, no collectives.

v2 changes vs baseline (301us):
  * x is shipped bf16 from the host (halves input DMA, 2x faster LN ops).
  * exp is split across ScalarE (exact, activation Exp) and VectorE
    (Schraudolph bit-hack: round(s*128*log2e + 16250.5) as int16 IS the
    bf16 pattern of ~e^s, one tensor_scalar op; ~3% elementwise, cancels
    through the softmax normalization).
  * LN rsqrt = exp(-0.5*ln(var+eps)) so every ScalarE activation stays in
    the natural_log_exp table set - the baseline paid ~14us of
    ACT_TABLE_LOAD thrash between sqrt and exp.
  * softmax normalize: one PSUM->SBUF copy of the whole [P,2,512] oT
    (rowsum rows included), DRAM-bounce reshape to [128,32] for the
    reciprocal, DRAM-bounce partition-broadcast back, one fused mul.
  * projection accumulates both regions in PSUM (no SBUF stash+add).
  * batch-0 prep-phase PSUM->SBUF copies run on the then-idle ScalarE.

Per-core dataflow (per batch):
  A. Load x row tiles [128, 1024] bf16, LayerNorm along free dim
     (bn_stats), normalize to bf16, transpose via PE matmul against a
     constant identity.
  B. q^T/k^T compact [128(f), n] via matmul with gamma/SCALE-folded
     weights, then SBUF->SBUF DMA relocation of each head's 16 rows to
     32-aligned "region" layout (4 heads per region at offsets 32c).
     v in row layout per (j-tile, head) as [128, 32]: col 0 = 1.0
     (softmax rowsum trick), cols 1..16 = v, rest 0.
  C. Per (r, ih, cp, jt): scores S^T[j,i] = k_h^T.T @ q_h^T (K=16,
     row-tiled via tile_position), exp (ScalarE or DVE per schedule),
     attn@v as oT[d,i] += v_aug.T @ E^T (K=128, col-tiled).  The ones
     column gives softmax row sums at oT row 32c; normalize as above.
     Row 32c becomes exactly 1.0; region 0 row 0 pairs with b_proj in
     w_proj_pad row 0 to add the bias for free.
  D. Projection with zero-padded w_proj rows.

Emission is software-pipelined across the 2 batches: batch b+1's
LN/qkv/v chunks and batch b's projection chunks are emitted between
attention groups of the current batch.
"""

from contextlib import ExitStack

import numpy as np
import ml_dtypes

import concourse.bass as bass
import concourse.tile as tile
from concourse import bacc, mybir
from concourse.bass_utils import run_bass_kernel_spmd

F32 = mybir.dt.float32
BF16 = mybir.dt.bfloat16
I16 = mybir.dt.int16

B, N, EMB = 16, 1024, 1024
HEADS, INNER = 8, 128
HD = INNER // HEADS            # 16
SCALE = INNER ** -0.5
EPS = 1e-5
NCORES = 8
NB = B // NCORES               # batches per core
P = 128
NT = EMB // P                  # 8 tiles along emb / n

Sub = mybir.AluOpType.subtract
Mult = mybir.AluOpType.mult
Add = mybir.AluOpType.add
AF = mybir.ActivationFunctionType

K1 = 128 * 1.4426950408889634        # schraudolph scale
K2 = 16256.0 - 5.5                   # schraudolph bias (HW rounds to nearest)

_CACHE = {}


def _dve_exp(r, ih, cp, jt):
    """Which attention groups compute exp on VectorE (Schraudolph)."""
    return jt == 3 or (jt == 7 and cp == 1)


def _build():
    nc = bacc.Bacc(None, target_bir_lowering=False)

    xs_h = nc.declare_dram_parameter("xs", [NB, N, EMB], BF16, isOutput=False)
    wqk_h = nc.declare_dram_parameter("wqk", [P, NT, 2, P], BF16, isOutput=False)
    bqk_h = nc.declare_dram_parameter("bqk", [P, 2], F32, isOutput=False)
    wv_h = nc.declare_dram_parameter("wv", [P, NT, P], BF16, isOutput=False)
    bv_h = nc.declare_dram_parameter("bv", [1, P], BF16, isOutput=False)
    wpj_h = nc.declare_dram_parameter("wproj", [P, 2, EMB], BF16, isOutput=False)
    id_h = nc.declare_dram_parameter("ident", [P, P], BF16, isOutput=False)
    out_h = nc.declare_dram_parameter("out", [NB, N, EMB], F32, isOutput=True)

    with tile.TileContext(nc) as tc, ExitStack() as ctx:
        ent = ctx.enter_context
        const = ent(tc.tile_pool(name="const", bufs=1))
        xpool = ent(tc.tile_pool(name="xpool", bufs=3))
        stat = ent(tc.tile_pool(name="stat", bufs=8))
        xT_pool = ent(tc.tile_pool(name="xT", bufs=2))
        qk_pool = ent(tc.tile_pool(name="qk", bufs=2))
        v_pool = ent(tc.tile_pool(name="vp", bufs=2))
        e_pool = ent(tc.tile_pool(name="ep", bufs=4))
        o_pool = ent(tc.tile_pool(name="op", bufs=4))
        nrm_pool = ent(tc.tile_pool(name="nrm", bufs=2))
        fin_pool = ent(tc.tile_pool(name="fin", bufs=4))
        dram_pool = ent(tc.tile_pool(name="dsc", bufs=2, space="DRAM"))
        ps_small = ent(tc.tile_pool(name="pss", bufs=2, space="PSUM"))
        ps_sc = ent(tc.tile_pool(name="psc", bufs=2, space="PSUM"))
        ps_oT = ent(tc.tile_pool(name="pso", bufs=1, space="PSUM"))

        # ---- constants ----
        wqk_sb = const.tile([P, NT, 2, P], BF16)
        nc.sync.dma_start(out=wqk_sb, in_=wqk_h[:])
        bqk_sb = const.tile([P, 2], F32)
        nc.sync.dma_start(out=bqk_sb, in_=bqk_h[:])
        wv_sb = const.tile([P, NT, P], BF16)
        nc.sync.dma_start(out=wv_sb, in_=wv_h[:])
        bv_sb = const.tile([1, P], BF16)
        nc.sync.dma_start(out=bv_sb, in_=bv_h[:])
        wpj_sb = const.tile([P, 2, EMB], BF16)
        nc.sync.dma_start(out=wpj_sb, in_=wpj_h[:])
        id_sb = const.tile([P, P], BF16)
        nc.sync.dma_start(out=id_sb, in_=id_h[:])
        eps_sb = const.tile([P, 1], F32)
        nc.vector.memset(eps_sb, EPS)
        ones1_sb = const.tile([1, P], BF16)
        nc.vector.memset(ones1_sb, 1.0)

        st8 = {0: {}, 1: {}}   # per-batch live tiles

        def emit_ln(b, it):
            s = st8[b]
            if s.get("xT") is None:
                s["xT"] = xT_pool.tile([P, NT, N], BF16, tag="xTt",
                                       name=f"xT{b}")
                s["xn"] = [None] * NT
            xt = xpool.tile([P, EMB], BF16, tag="xt")
            nc.sync.dma_start(out=xt, in_=xs_h[b, it * P:(it + 1) * P, :])
            st = stat.tile([P, 2, 6], F32, tag="st")
            nc.vector.bn_stats(out=st[:, 0, :], in_=xt[:, 0:512])
            nc.vector.bn_stats(out=st[:, 1, :], in_=xt[:, 512:1024])
            mv = stat.tile([P, 2], F32, tag="mv")
            nc.vector.bn_aggr(out=mv, in_=st)
            # rs = 1/sqrt(var+eps) = exp(-0.5*ln(var+eps)); Ln+Exp live in
            # the same ACT table set so no table reloads ever happen.
            lnv = stat.tile([P, 1], F32, tag="lnv")
            nc.scalar.activation(out=lnv, in_=mv[:, 1:2], func=AF.Ln,
                                 bias=eps_sb)
            rs = stat.tile([P, 1], F32, tag="rs")
            nc.scalar.activation(out=rs, in_=lnv, func=AF.Exp, scale=-0.5)
            xn = xpool.tile([P, EMB], BF16, tag="xn")
            nc.vector.tensor_scalar(
                out=xn, in0=xt, scalar1=mv[:, 0:1], scalar2=rs,
                op0=Sub, op1=Mult)
            s["xn"][it] = xn

        def emit_tp(b, it):
            s = st8[b]
            xT = s["xT"]
            xn = s["xn"][it]
            ceng = nc.scalar if b == 0 else nc.vector
            for eg in range(2):
                tp = ps_small.tile([P, 4, P], F32, tag="smallps")
                for kk in range(4):
                    et = 4 * eg + kk
                    nc.tensor.matmul(
                        tp[:, kk, :], xn[:, et * P:(et + 1) * P], id_sb,
                        start=True, stop=True)
                if b == 0:
                    nc.scalar.copy(
                        out=xT[:, 4 * eg:4 * eg + 4, it * P:(it + 1) * P],
                        in_=tp)
                else:
                    nc.vector.tensor_copy(
                        out=xT[:, 4 * eg:4 * eg + 4, it * P:(it + 1) * P],
                        in_=tp)

        def emit_qk(b, t, nt):
            # compact q^T/k^T halves; on the last nt of each t, relocate
            # head rows into the 32-aligned region layout.
            s = st8[b]
            if s.get("qkc") is None:
                s["qkc"] = qk_pool.tile([P, 2, N], BF16, tag="qkc",
                                        name=f"qkc{b}")
                s["qT"] = qk_pool.tile([P, 2, N], BF16, tag="qT",
                                       name=f"qT{b}")
                s["kT"] = qk_pool.tile([P, 2, N], BF16, tag="kT",
                                       name=f"kT{b}")
            xT = s["xT"]
            ps = ps_small.tile([P, 512], F32, tag="smallps")
            for et in range(NT):
                nc.tensor.matmul(
                    ps, wqk_sb[:, et, t, :],
                    xT[:, et, nt * 512:(nt + 1) * 512],
                    start=(et == 0), stop=(et == NT - 1))
            nc.vector.tensor_scalar(
                out=s["qkc"][:, t, nt * 512:(nt + 1) * 512], in0=ps,
                scalar1=bqk_sb[:, t:t + 1], scalar2=None, op0=Add)
            if nt == 1:
                dst = s["qT"] if t == 0 else s["kT"]
                eng = nc.scalar if b == 0 else nc.sync
                for h in range(HEADS):
                    r, c = h // 4, h % 4
                    eng.dma_start(
                        out=dst[32 * c:32 * c + HD, r, :],
                        in_=s["qkc"][HD * h:HD * (h + 1), t, :])

        def emit_v(b, jt):
            s = st8[b]
            if s.get("v") is None:
                s["v"] = v_pool.tile([P, NT, HEADS, 32], BF16, tag="vt",
                                     name=f"v{b}")
                nc.gpsimd.memset(s["v"], 0.0)
                nc.gpsimd.memset(s["v"][:, :, :, 0:1], 1.0)
            xT = s["xT"]
            ps = ps_small.tile([P, P], F32, tag="smallps")
            for et in range(NT):
                nc.tensor.matmul(
                    ps, xT[:, et, jt * P:(jt + 1) * P], wv_sb[:, et, :],
                    start=(et == 0), stop=False)
            nc.tensor.matmul(ps, ones1_sb, bv_sb, start=False, stop=True)
            src = ps[:].rearrange("p (h d) -> p h d", d=16)
            if b == 0:
                nc.scalar.copy(out=s["v"][:, jt, :, 1:17], in_=src)
            else:
                nc.vector.tensor_copy(out=s["v"][:, jt, :, 1:17], in_=src)

        def emit_proj(b, it, nt, ceng):
            s = st8[b]
            ps = ps_small.tile([P, 512], F32, tag="smallps")
            for r in range(2):
                nc.tensor.matmul(
                    ps, s["o"][r][:, it * P:(it + 1) * P],
                    wpj_sb[:, r, nt * 512:(nt + 1) * 512],
                    start=(r == 0), stop=(r == 1))
            fin = fin_pool.tile([P, 512], F32, tag="fin")
            if ceng == "s":
                nc.scalar.copy(out=fin, in_=ps)
            else:
                nc.vector.tensor_copy(out=fin, in_=ps)
            nc.sync.dma_start(
                out=out_h[b, it * P:(it + 1) * P, nt * 512:(nt + 1) * 512],
                in_=fin)

        def emit_normalize(b, r, oT_ps):
            # oT_ps [P, 2, 512] f32: rows 32c = rowsums, rows 32c+1+d =
            # head (4r+c) outputs.  Divide every row of band c by the
            # band's rowsum (per free position).
            s = st8[b]
            t_sb = nrm_pool.tile([P, 2, 512], BF16, tag="tsb")
            nc.vector.tensor_copy(out=t_sb, in_=oT_ps)
            scr1 = dram_pool.tile([4, 2, 512], BF16, tag="scr1")
            nc.sync.dma_start(out=scr1, in_=t_sb[0::32, :, :])
            cmp = nrm_pool.tile([P, 32], BF16, tag="cmp")
            flat = scr1[:].rearrange("c h (pp cc) -> (c h pp) cc", cc=32)
            nc.sync.dma_start(out=cmp, in_=flat)
            rec = nrm_pool.tile([P, 32], BF16, tag="rec")
            with nc.allow_low_precision(reason="bf16 softmax rowsum recip"):
                nc.vector.reciprocal(out=rec, in_=cmp)
            scr2 = dram_pool.tile([4, N], BF16, tag="scr2")
            nc.sync.dma_start(
                out=scr2[:].rearrange("c (h pp cc) -> (c h pp) cc", cc=32,
                                      h=2),
                in_=rec)
            rep = nrm_pool.tile([P, N], BF16, tag="rep")
            for c in range(4):
                src = scr2[c:c + 1, :]
                bcast = bass.AP(
                    tensor=src.tensor, offset=src.offset,
                    ap=[[0, 32]] + list(src.ap[1:]))
                nc.sync.dma_start(
                    out=rep[32 * c:32 * c + 32, :], in_=bcast)
            o_r = o_pool.tile([P, N], BF16, tag="oT", name=f"o{b}{r}")
            nc.vector.tensor_tensor(
                out=o_r[:].rearrange("p (h i) -> p h i", h=2),
                in0=t_sb, in1=rep[:].rearrange("p (h i) -> p h i", h=2),
                op=Mult)
            s["o"][r] = o_r

        def emit_attention(b, fillers, rate=2):
            s = st8[b]
            s["o"] = [None, None]
            slot = [0]

            def maybe_fill():
                slot[0] += 1
                if fillers and slot[0] % rate == 0:
                    f = fillers.pop(0)
                    if f is not None:
                        f()

            for r in range(2):
                oT_ps = ps_oT.tile([P, 2, 512], F32, tag="oTps")
                for ih in range(2):
                    i0 = ih * 512
                    for cp in range(2):
                        c0 = 2 * cp
                        for jt in range(NT):
                            E = e_pool.tile([P, 2, 512], BF16, tag="E")
                            sc = ps_sc.tile([P, 2, 512], F32, tag="sc")
                            for ci in range(2):
                                c = c0 + ci
                                nc.tensor.matmul(
                                    sc[:, ci, :],
                                    s["kT"][32 * c:32 * c + 16, r,
                                            jt * P:(jt + 1) * P],
                                    s["qT"][32 * c:32 * c + 16, r,
                                            i0:i0 + 512],
                                    start=True, stop=True,
                                    tile_position=(32 * c, 0))
                            if _dve_exp(r, ih, cp, jt):
                                nc.vector.tensor_scalar(
                                    out=E[:].bitcast(I16), in0=sc,
                                    scalar1=K1, scalar2=K2,
                                    op0=Mult, op1=Add)
                            else:
                                nc.scalar.activation(out=E, in_=sc,
                                                     func=AF.Exp)
                            for ci in range(2):
                                c = c0 + ci
                                h = 4 * r + c
                                nc.tensor.matmul(
                                    oT_ps[32 * c:32 * c + 32, ih, :],
                                    s["v"][:, jt, h, :], E[:, ci, :],
                                    start=(jt == 0), stop=(jt == NT - 1),
                                    tile_position=(0, 32 * c))
                            maybe_fill()
                emit_normalize(b, r, oT_ps)

        # ---------- schedule ----------
        # preload the natural_log_exp table set while the DMA ramp runs
        dummy = stat.tile([P, 1], F32, tag="dummy")
        nc.scalar.activation(out=dummy, in_=eps_sb, func=AF.Ln)
        nc.scalar.activation(out=dummy, in_=dummy, func=AF.Exp)

        def ab_order(b):
            out = []
            for it in range(4):
                out.append(lambda it=it: emit_ln(b, it))
                out.append(lambda it=it: emit_tp(b, it))
                out.append(lambda it=it: emit_v(b, it))
            out.append(lambda: emit_qk(b, 0, 0))
            out.append(lambda: emit_qk(b, 1, 0))
            for it in range(4, NT):
                out.append(lambda it=it: emit_ln(b, it))
                out.append(lambda it=it: emit_tp(b, it))
                out.append(lambda it=it: emit_v(b, it))
            out.append(lambda: emit_qk(b, 0, 1))
            out.append(lambda: emit_qk(b, 1, 1))
            return out

        for f in ab_order(0):
            f()

        fill_b1 = ab_order(1)
        emit_attention(0, fill_b1, rate=2)
        for f in fill_b1:
            f()

        fill_p0 = [lambda it=it, nt=nt: emit_proj(0, it, nt, "v")
                   for it in range(NT) for nt in range(2)]
        emit_attention(1, fill_p0, rate=4)
        for f in fill_p0:
            f()

        for it in range(NT):
            for nt in range(2):
                emit_proj(1, it, nt, "s" if (it + nt) % 2 else "v")

    nc.finalize()
    return nc


def _prep_weights(gamma, beta, w_qkv, w_proj, b_proj):
    gamma = gamma.astype(np.float64)
    beta = beta.astype(np.float64)
    w_qkv = w_qkv.astype(np.float64)
    w_proj = w_proj.astype(np.float64)
    b_proj = b_proj.astype(np.float64)

    wg = w_qkv * gamma[:, None]
    bias = beta @ w_qkv                   # [384]

    # compact q/k: tile t=0 -> q (SCALE folded), t=1 -> k
    wqk = np.zeros((EMB, 2, P), dtype=np.float64)
    wqk[:, 0, :] = wg[:, :INNER] * SCALE
    wqk[:, 1, :] = wg[:, INNER:2 * INNER]
    bqk = np.zeros((P, 2), dtype=np.float64)
    bqk[:, 0] = bias[:INNER] * SCALE
    bqk[:, 1] = bias[INNER:2 * INNER]
    wqk_t = wqk.reshape(NT, P, 2, P).transpose(1, 0, 2, 3)  # [P, NT, 2, P]

    wv = wg[:, 2 * INNER:3 * INNER].reshape(NT, P, P).transpose(1, 0, 2)
    bv = bias[2 * INNER:3 * INNER].reshape(1, P)

    # o^T row mapping: 32c = ones/rowsum row, 32c+1+d = head (4r+c) dim d
    wpj = np.zeros((P, 2, EMB), dtype=np.float64)
    for r in range(2):
        for c in range(4):
            h = 4 * r + c
            wpj[32 * c + 1:32 * c + 1 + HD, r, :] = \
                w_proj[h * HD:(h + 1) * HD, :]
    wpj[0, 0, :] = b_proj

    bf = ml_dtypes.bfloat16
    return {
        "wqk": np.ascontiguousarray(wqk_t).astype(bf),
        "bqk": np.ascontiguousarray(bqk).astype(np.float32),
        "wv": np.ascontiguousarray(wv).astype(bf),
        "bv": np.ascontiguousarray(bv).astype(bf),
        "wproj": np.ascontiguousarray(wpj).astype(bf),
        "ident": np.eye(P, dtype=np.float32).astype(bf),
    }


def kernel(x, gamma, beta, w_qkv, w_proj, b_proj):
    if "nc" not in _CACHE:
        _CACHE["nc"] = _build()
    nc = _CACHE["nc"]

    w = _prep_weights(gamma, beta, w_qkv, w_proj, b_proj)
    xb = np.asarray(x, dtype=np.float32).astype(ml_dtypes.bfloat16)
    in_maps = []
    for i in range(NCORES):
        m = {"xs": np.ascontiguousarray(xb[i * NB:(i + 1) * NB])}
        m.update(w)
        in_maps.append(m)

    res = run_bass_kernel_spmd(nc, in_maps, core_ids=list(range(NCORES)))
    out = np.concatenate([res.results[i]["out"] for i in range(NCORES)], axis=0)
    return out.astype(np.float32)


# revision 10
# speedup vs baseline: 1.1079x; 1.1079x over previous
"""Trainium2 Bass kernel for nn_Attention_83004537963197.

LayerNorm -> QKV projection -> 8-head attention (head_dim=16) -> output
projection, x[16, 1024, 1024] f32.  Data-parallel over batch: 2 batches
per NeuronCore across 8 cores, no collectives.

v2 changes vs baseline (301us):
  * x is shipped bf16 from the host (halves input DMA, 2x faster LN ops).
  * exp is split across ScalarE (exact, activation Exp) and VectorE
    (Schraudolph bit-hack: round(s*128*log2e + 16250.5) as int16 IS the
    bf16 pattern of ~e^s, one tensor_scalar op; ~3% elementwise, cancels
    through the softmax normalization).
  * LN rsqrt = exp(-0.5*ln(var+eps)) so every ScalarE activation stays in
    the natural_log_exp table set - the baseline paid ~14us of
    ACT_TABLE_LOAD thrash between sqrt and exp.
  * softmax normalize: one PSUM->SBUF copy of the whole [P,2,512] oT
    (rowsum rows included), DRAM-bounce reshape to [128,32] for the
    reciprocal, DRAM-bounce partition-broadcast back, one fused mul.
  * projection accumulates both regions in PSUM (no SBUF stash+add).
  * batch-0 prep-phase PSUM->SBUF copies run on the then-idle ScalarE.

Per-core dataflow (per batch):
  A. Load x row tiles [128, 1024] bf16, LayerNorm along free dim
     (bn_stats), normalize to bf16, transpose via PE matmul against a
     constant identity.
  B. q^T/k^T compact [128(f), n] via matmul with gamma/SCALE-folded
     weights, then SBUF->SBUF DMA relocation of each head's 16 rows to
     32-aligned "region" layout (4 heads per region at offsets 32c).
     v in row layout per (j-tile, head) as [128, 32]: col 0 = 1.0
     (softmax rowsum trick), cols 1..16 = v, rest 0.
  C. Per (r, ih, cp, jt): scores S^T[j,i] = k_h^T.T @ q_h^T (K=16,
     row-tiled via tile_position), exp (ScalarE or DVE per schedule),
     attn@v as oT[d,i] += v_aug.T @ E^T (K=128, col-tiled).  The ones
     column gives softmax row sums at oT row 32c; normalize as above.
     Row 32c becomes exactly 1.0; region 0 row 0 pairs with b_proj in
     w_proj_pad row 0 to add the bias for free.
  D. Projection with zero-padded w_proj rows.

Emission is software-pipelined across the 2 batches: batch b+1's
LN/qkv/v chunks and batch b's projection chunks are emitted between
attention groups of the current batch.
"""

from contextlib import ExitStack

import numpy as np
import ml_dtypes

import concourse.bass as bass
import concourse.tile as tile
from concourse import bacc, mybir
from concourse.bass_utils import run_bass_kernel_spmd

F32 = mybir.dt.float32
BF16 = mybir.dt.bfloat16
I16 = mybir.dt.int16

B, N, EMB = 16, 1024, 1024
HEADS, INNER = 8, 128
HD = INNER // HEADS            # 16
SCALE = INNER ** -0.5
EPS = 1e-5
NCORES = 8
NB = B // NCORES               # batches per core
P = 128
NT = EMB // P                  # 8 tiles along emb / n

Sub = mybir.AluOpType.subtract
Mult = mybir.AluOpType.mult
Add = mybir.AluOpType.add
AF = mybir.ActivationFunctionType

K1 = 128 * 1.4426950408889634        # schraudolph scale
K2 = 16256.0 - 5.5                   # schraudolph bias (HW rounds to nearest)
RSQRT_MAGIC = 0x5f3759df
I32 = mybir.dt.int32

_CACHE = {}


def _dve_exp(b, r, ih, cp, jt):
    """Which attention groups compute exp on VectorE (Schraudolph)."""
    if b == 0:
        return jt == 3 or (jt == 7 and cp == 1)
    return jt in (3, 7)


def _build():
    nc = bacc.Bacc(None, target_bir_lowering=False)

    xs_h = nc.declare_dram_parameter("xs", [NB, N, EMB], BF16, isOutput=False)
    wqk_h = nc.declare_dram_parameter("wqk", [P, NT, 2, P], BF16, isOutput=False)
    bqk_h = nc.declare_dram_parameter("bqk", [P, 2], F32, isOutput=False)
    wv_h = nc.declare_dram_parameter("wv", [P, NT, P], BF16, isOutput=False)
    bv_h = nc.declare_dram_parameter("bv", [1, P], BF16, isOutput=False)
    wpj_h = nc.declare_dram_parameter("wproj", [P, 2, EMB], BF16, isOutput=False)
    id_h = nc.declare_dram_parameter("ident", [P, P], BF16, isOutput=False)
    out_h = nc.declare_dram_parameter("out", [NB, N, EMB], F32, isOutput=True)

    with tile.TileContext(nc) as tc, ExitStack() as ctx:
        ent = ctx.enter_context
        const = ent(tc.tile_pool(name="const", bufs=1))
        xpool = ent(tc.tile_pool(name="xpool", bufs=6))
        stat = ent(tc.tile_pool(name="stat", bufs=8))
        xT_pool = ent(tc.tile_pool(name="xT", bufs=2))
        qk_pool = ent(tc.tile_pool(name="qk", bufs=2))
        v_pool = ent(tc.tile_pool(name="vp", bufs=2))
        e_pool = ent(tc.tile_pool(name="ep", bufs=4))
        o_pool = ent(tc.tile_pool(name="op", bufs=4))
        nrm_pool = ent(tc.tile_pool(name="nrm", bufs=2))
        fin_pool = ent(tc.tile_pool(name="fin", bufs=4))
        dram_pool = ent(tc.tile_pool(name="dsc", bufs=2, space="DRAM"))
        ps_small = ent(tc.tile_pool(name="pss", bufs=2, space="PSUM"))
        ps_sc = ent(tc.tile_pool(name="psc", bufs=2, space="PSUM"))
        ps_oT = ent(tc.tile_pool(name="pso", bufs=2, space="PSUM"))

        # ---- constants ----
        wqk_sb = const.tile([P, NT, 2, P], BF16)
        nc.sync.dma_start(out=wqk_sb, in_=wqk_h[:])
        bqk_sb = const.tile([P, 2], F32)
        nc.sync.dma_start(out=bqk_sb, in_=bqk_h[:])
        wv_sb = const.tile([P, NT, P], BF16)
        nc.sync.dma_start(out=wv_sb, in_=wv_h[:])
        bv_sb = const.tile([1, P], BF16)
        nc.sync.dma_start(out=bv_sb, in_=bv_h[:])
        wpj_sb = const.tile([P, 2, EMB], BF16)
        nc.sync.dma_start(out=wpj_sb, in_=wpj_h[:])
        id_sb = const.tile([P, P], BF16)
        nc.sync.dma_start(out=id_sb, in_=id_h[:])
        eps_sb = const.tile([P, 1], F32)
        nc.vector.memset(eps_sb, EPS)
        ones1_sb = const.tile([1, P], BF16)
        nc.vector.memset(ones1_sb, 1.0)

        st8 = {0: {}, 1: {}}   # per-batch live tiles

        def emit_stat(b, it):
            s = st8[b]
            if s.get("xT") is None:
                s["xT"] = xT_pool.tile([P, NT, N], BF16, tag="xTt",
                                       name=f"xT{b}")
                s["xn"] = [None] * NT
                s["xt"] = [None] * NT
                s["mv"] = stat.tile([P, NT, 2], F32, tag="mvall",
                                    name=f"mv{b}")
                s["rs"] = [None, None]
            xt = xpool.tile([P, EMB], BF16, tag="xt")
            nc.sync.dma_start(out=xt, in_=xs_h[b, it * P:(it + 1) * P, :])
            st = stat.tile([P, 2, 6], F32, tag="st")
            nc.vector.bn_stats(out=st[:, 0, :], in_=xt[:, 0:512])
            nc.vector.bn_stats(out=st[:, 1, :], in_=xt[:, 512:1024])
            nc.vector.bn_aggr(out=s["mv"][:, it, :], in_=st)
            s["xt"][it] = xt

        def emit_rsqrt(b, h):
            # rs[4h..4h+3] = 1/sqrt(var+eps) entirely on the DVE:
            # fast-inverse-sqrt bit hack + 2 Newton steps, [P, 4] wide.
            s = st8[b]
            var = s["mv"][:, 4 * h:4 * h + 4, 1:2]
            ve = stat.tile([P, 4], F32, tag="ve")
            nc.vector.tensor_scalar(out=ve, in0=var, scalar1=EPS,
                                    scalar2=None, op0=Add)
            iv = stat.tile([P, 4], I32, tag="iv")
            nc.vector.tensor_scalar(out=iv, in0=ve[:].bitcast(I32),
                                    scalar1=1, scalar2=None,
                                    op0=mybir.AluOpType.arith_shift_right)
            y0 = stat.tile([P, 4], I32, tag="y0")
            nc.vector.tensor_scalar(out=y0, in0=iv, scalar1=-1,
                                    scalar2=float(RSQRT_MAGIC),
                                    op0=Mult, op1=Add)
            y = y0[:].bitcast(F32)
            for itn in range(2):
                t = stat.tile([P, 4], F32, tag=f"nt{itn}")
                nc.vector.tensor_tensor(out=t, in0=y, in1=y, op=Mult)
                nc.vector.tensor_tensor(out=t, in0=t, in1=ve, op=Mult)
                nc.vector.tensor_scalar(out=t, in0=t, scalar1=-0.5,
                                        scalar2=1.5, op0=Mult, op1=Add)
                yn = stat.tile([P, 4], F32, tag=f"ny{itn}")
                nc.vector.tensor_tensor(out=yn, in0=y, in1=t, op=Mult)
                y = yn
            s["rs"][h] = y

        def emit_ln_apply(b, it):
            s = st8[b]
            xt = s["xt"][it]
            rs = s["rs"][it // 4][:, (it % 4):(it % 4) + 1]
            xn = xpool.tile([P, EMB], BF16, tag="xn")
            nc.vector.tensor_scalar(
                out=xn, in0=xt, scalar1=s["mv"][:, it, 0:1], scalar2=rs,
                op0=Sub, op1=Mult)
            s["xn"][it] = xn
            s["xt"][it] = None

        def emit_tp(b, it):
            s = st8[b]
            xT = s["xT"]
            xn = s["xn"][it]
            ceng = nc.scalar if b == 0 else nc.vector
            for eg in range(2):
                tp = ps_small.tile([P, 4, P], F32, tag="smallps")
                for kk in range(4):
                    et = 4 * eg + kk
                    nc.tensor.matmul(
                        tp[:, kk, :], xn[:, et * P:(et + 1) * P], id_sb,
                        start=True, stop=True)
                if b == 0:
                    nc.scalar.copy(
                        out=xT[:, 4 * eg:4 * eg + 4, it * P:(it + 1) * P],
                        in_=tp)
                else:
                    nc.vector.tensor_copy(
                        out=xT[:, 4 * eg:4 * eg + 4, it * P:(it + 1) * P],
                        in_=tp)

        def emit_qk(b, t, nt):
            # compact q^T/k^T halves; on the last nt of each t, relocate
            # head rows into the 32-aligned region layout.
            s = st8[b]
            if s.get("qkc") is None:
                s["qkc"] = qk_pool.tile([P, 2, N], BF16, tag="qkc",
                                        name=f"qkc{b}")
                s["qT"] = qk_pool.tile([P, 2, N], BF16, tag="qT",
                                       name=f"qT{b}")
                s["kT"] = qk_pool.tile([P, 2, N], BF16, tag="kT",
                                       name=f"kT{b}")
            xT = s["xT"]
            ps = ps_small.tile([P, 512], F32, tag="smallps")
            for et in range(NT):
                nc.tensor.matmul(
                    ps, wqk_sb[:, et, t, :],
                    xT[:, et, nt * 512:(nt + 1) * 512],
                    start=(et == 0), stop=(et == NT - 1))
            nc.vector.tensor_scalar(
                out=s["qkc"][:, t, nt * 512:(nt + 1) * 512], in0=ps,
                scalar1=bqk_sb[:, t:t + 1], scalar2=None, op0=Add)
            if nt == 1:
                dst = s["qT"] if t == 0 else s["kT"]
                eng = nc.scalar if b == 0 else nc.sync
                for h in range(HEADS):
                    r, c = h // 4, h % 4
                    eng.dma_start(
                        out=dst[32 * c:32 * c + HD, r, :],
                        in_=s["qkc"][HD * h:HD * (h + 1), t, :])

        def emit_v(b, jt):
            s = st8[b]
            if s.get("v") is None:
                s["v"] = v_pool.tile([P, NT, HEADS, 32], BF16, tag="vt",
                                     name=f"v{b}")
                nc.gpsimd.memset(s["v"], 0.0)
                nc.gpsimd.memset(s["v"][:, :, :, 0:1], 1.0)
            xT = s["xT"]
            ps = ps_small.tile([P, P], F32, tag="smallps")
            for et in range(NT):
                nc.tensor.matmul(
                    ps, xT[:, et, jt * P:(jt + 1) * P], wv_sb[:, et, :],
                    start=(et == 0), stop=False)
            nc.tensor.matmul(ps, ones1_sb, bv_sb, start=False, stop=True)
            src = ps[:].rearrange("p (h d) -> p h d", d=16)
            if b == 0:
                nc.scalar.copy(out=s["v"][:, jt, :, 1:17], in_=src)
            else:
                nc.vector.tensor_copy(out=s["v"][:, jt, :, 1:17], in_=src)

        def emit_proj(b, it, nt, ceng):
            s = st8[b]
            ps = ps_small.tile([P, 512], F32, tag="smallps")
            for r in range(2):
                nc.tensor.matmul(
                    ps, s["o"][r][:, it * P:(it + 1) * P],
                    wpj_sb[:, r, nt * 512:(nt + 1) * 512],
                    start=(r == 0), stop=(r == 1))
            fin = fin_pool.tile([P, 512], F32, tag="fin")
            if ceng == "s":
                nc.scalar.copy(out=fin, in_=ps)
            else:
                nc.vector.tensor_copy(out=fin, in_=ps)
            nc.sync.dma_start(
                out=out_h[b, it * P:(it + 1) * P, nt * 512:(nt + 1) * 512],
                in_=fin)

        def emit_normalize(b, r, ih, oT_ps):
            # oT_ps [P, 512] f32: rows 32c = rowsums, rows 32c+1+d = head
            # (4r+c) outputs.  Divide every row of band c by the band's
            # rowsum (per free position).
            s = st8[b]
            if s["o"][r] is None:
                s["o"][r] = o_pool.tile([P, N], BF16, tag="oT",
                                        name=f"o{b}{r}")
            t_sb = nrm_pool.tile([P, 512], BF16, tag="tsb")
            nc.vector.tensor_copy(out=t_sb, in_=oT_ps)
            scr1 = dram_pool.tile([4, 512], BF16, tag="scr1")
            nc.sync.dma_start(out=scr1, in_=t_sb[0::32, :])
            cmp = nrm_pool.tile([P, 16], BF16, tag="cmp")
            flat = scr1[:].rearrange("c (pp cc) -> (c pp) cc", cc=16)
            nc.sync.dma_start(out=cmp, in_=flat)
            rec = nrm_pool.tile([P, 16], BF16, tag="rec")
            with nc.allow_low_precision(reason="bf16 softmax rowsum recip"):
                nc.vector.reciprocal(out=rec, in_=cmp)
            scr2 = dram_pool.tile([4, 512], BF16, tag="scr2")
            nc.sync.dma_start(
                out=scr2[:].rearrange("c (pp cc) -> (c pp) cc", cc=16),
                in_=rec)
            rep = nrm_pool.tile([P, 512], BF16, tag="rep")
            for c in range(4):
                src = scr2[c:c + 1, :]
                bcast = bass.AP(
                    tensor=src.tensor, offset=src.offset,
                    ap=[[0, 32]] + list(src.ap[1:]))
                nc.sync.dma_start(
                    out=rep[32 * c:32 * c + 32, :], in_=bcast)
            i0 = ih * 512
            nc.vector.tensor_tensor(
                out=s["o"][r][:, i0:i0 + 512],
                in0=t_sb, in1=rep, op=Mult)

        def emit_attention(b, fillers, rate=2):
            # software-pipelined by one group: scores+exp of group g+1 are
            # emitted before attn@v of group g, so the in-order PE queue
            # never head-of-line-blocks on the exp result.
            s = st8[b]
            s["o"] = [None, None]
            slot = [0]

            def maybe_fill():
                slot[0] += 1
                if fillers and slot[0] % rate == 0:
                    f = fillers.pop(0)
                    if f is not None:
                        f()

            def scores_exp(r, ih, cp, jt):
                i0 = ih * 512
                E = e_pool.tile([P, 2, 512], BF16, tag="E")
                sc = ps_sc.tile([P, 2, 512], F32, tag="sc")
                for ci in range(2):
                    c = 2 * cp + ci
                    nc.tensor.matmul(
                        sc[:, ci, :],
                        s["kT"][32 * c:32 * c + 16, r,
                                jt * P:(jt + 1) * P],
                        s["qT"][32 * c:32 * c + 16, r, i0:i0 + 512],
                        start=True, stop=True,
                        tile_position=(32 * c, 0))
                if _dve_exp(b, r, ih, cp, jt):
                    nc.vector.tensor_scalar(
                        out=E[:].bitcast(I16), in0=sc,
                        scalar1=K1, scalar2=K2, op0=Mult, op1=Add)
                else:
                    nc.scalar.activation(out=E, in_=sc, func=AF.Exp)
                return E

            def attnv(r, ih, cp, jt, E, oT_ps):
                for ci in range(2):
                    c = 2 * cp + ci
                    h = 4 * r + c
                    nc.tensor.matmul(
                        oT_ps[32 * c:32 * c + 32, :],
                        s["v"][:, jt, h, :], E[:, ci, :],
                        start=(jt == 0), stop=(jt == NT - 1),
                        tile_position=(0, 32 * c))

            prev = None
            for r in range(2):
                for ih in range(2):
                    oT_ps = ps_oT.tile([P, 512], F32, tag="oTps")
                    for cp in range(2):
                        for jt in range(NT):
                            E = scores_exp(r, ih, cp, jt)
                            if prev is not None:
                                attnv(*prev)
                                if prev[1] != ih or prev[0] != r:
                                    emit_normalize(b, prev[0], prev[1],
                                                   prev[5])
                            prev = (r, ih, cp, jt, E, oT_ps)
                            maybe_fill()
            attnv(*prev)
            emit_normalize(b, prev[0], prev[1], prev[5])

        # ---------- schedule ----------
        # preload the exp table set while the DMA ramp runs
        dummy = stat.tile([P, 1], F32, tag="dummy")
        nc.scalar.activation(out=dummy, in_=eps_sb, func=AF.Exp)

        def ab_order(b):
            out = []
            for it in range(4):
                out.append(lambda it=it: emit_stat(b, it))
            out.append(lambda: emit_rsqrt(b, 0))
            for it in range(4):
                out.append(lambda it=it: emit_ln_apply(b, it))
                out.append(lambda it=it: emit_tp(b, it))
                out.append(lambda it=it: emit_v(b, it))
                if it < 4:
                    out.append(lambda it=it: emit_stat(b, it + 4))
            out.append(lambda: emit_qk(b, 0, 0))
            out.append(lambda: emit_qk(b, 1, 0))
            out.append(lambda: emit_rsqrt(b, 1))
            for it in range(4, NT):
                out.append(lambda it=it: emit_ln_apply(b, it))
                out.append(lambda it=it: emit_tp(b, it))
                out.append(lambda it=it: emit_v(b, it))
            out.append(lambda: emit_qk(b, 0, 1))
            out.append(lambda: emit_qk(b, 1, 1))
            return out

        for f in ab_order(0):
            f()

        fill_b1 = ab_order(1)
        emit_attention(0, fill_b1, rate=2)
        for f in fill_b1:
            f()

        fill_p0 = [lambda it=it, nt=nt: emit_proj(0, it, nt, "v")
                   for it in range(NT) for nt in range(2)]
        emit_attention(1, fill_p0, rate=4)
        for f in fill_p0:
            f()

        for it in range(NT):
            for nt in range(2):
                emit_proj(1, it, nt, "s" if (it + nt) % 2 else "v")

    nc.finalize()
    return nc


def _prep_weights(gamma, beta, w_qkv, w_proj, b_proj):
    gamma = gamma.astype(np.float64)
    beta = beta.astype(np.float64)
    w_qkv = w_qkv.astype(np.float64)
    w_proj = w_proj.astype(np.float64)
    b_proj = b_proj.astype(np.float64)

    wg = w_qkv * gamma[:, None]
    bias = beta @ w_qkv                   # [384]

    # compact q/k: tile t=0 -> q (SCALE folded), t=1 -> k
    wqk = np.zeros((EMB, 2, P), dtype=np.float64)
    wqk[:, 0, :] = wg[:, :INNER] * SCALE
    wqk[:, 1, :] = wg[:, INNER:2 * INNER]
    bqk = np.zeros((P, 2), dtype=np.float64)
    bqk[:, 0] = bias[:INNER] * SCALE
    bqk[:, 1] = bias[INNER:2 * INNER]
    wqk_t = wqk.reshape(NT, P, 2, P).transpose(1, 0, 2, 3)  # [P, NT, 2, P]

    wv = wg[:, 2 * INNER:3 * INNER].reshape(NT, P, P).transpose(1, 0, 2)
    bv = bias[2 * INNER:3 * INNER].reshape(1, P)

    # o^T row mapping: 32c = ones/rowsum row, 32c+1+d = head (4r+c) dim d
    wpj = np.zeros((P, 2, EMB), dtype=np.float64)
    for r in range(2):
        for c in range(4):
            h = 4 * r + c
            wpj[32 * c + 1:32 * c + 1 + HD, r, :] = \
                w_proj[h * HD:(h + 1) * HD, :]
    wpj[0, 0, :] = b_proj

    bf = ml_dtypes.bfloat16
    return {
        "wqk": np.ascontiguousarray(wqk_t).astype(bf),
        "bqk": np.ascontiguousarray(bqk).astype(np.float32),
        "wv": np.ascontiguousarray(wv).astype(bf),
        "bv": np.ascontiguousarray(bv).astype(bf),
        "wproj": np.ascontiguousarray(wpj).astype(bf),
        "ident": np.eye(P, dtype=np.float32).astype(bf),
    }


def kernel(x, gamma, beta, w_qkv, w_proj, b_proj):
    if "nc" not in _CACHE:
        _CACHE["nc"] = _build()
    nc = _CACHE["nc"]

    w = _prep_weights(gamma, beta, w_qkv, w_proj, b_proj)
    xb = np.asarray(x, dtype=np.float32).astype(ml_dtypes.bfloat16)
    in_maps = []
    for i in range(NCORES):
        m = {"xs": np.ascontiguousarray(xb[i * NB:(i + 1) * NB])}
        m.update(w)
        in_maps.append(m)

    res = run_bass_kernel_spmd(nc, in_maps, core_ids=list(range(NCORES)))
    out = np.concatenate([res.results[i]["out"] for i in range(NCORES)], axis=0)
    return out.astype(np.float32)
